# revision 2
# baseline (speedup 1.0000x reference)
"""Single-head causal attention with RoPE on 8 TRN2 NeuronCores — v3.

Sharding: core c -> batch c//2, parity p = c%2 takes the interleaved
512-row q-blocks {p, p+2, p+4, p+6} of T=4096. Full K/V per core.

v3 vs v2 (184us):
- Startup: weights DMA'd first; x loaded as 4 separate 512KB tiles per
  t-group so the first projection matmul only waits on ~1MB.
- Phase 2: S tiles/exps batched in [128, 1024] pairs (halves the ACT
  fixed cost); probability row-sums via bf16 pair-sum on GpSimd +
  bf16 accumulate on DVE + one ones-matmul per q-block; o2/sm copies
  stay on ACT. V path identical to v2 (PE transpose).
"""
import numpy as np
import ml_dtypes

B, T, C, HD = 4, 4096, 2048, 128
P = 128
NB = 8
BS = 512
NCH = 16
SCALE = float(C) ** -0.5
NEG = -1.0e9
BF = ml_dtypes.bfloat16


def build():
    import concourse.bass as bass
    import concourse.mybir as mybir
    import bass_rust
    from concourse.tile import TileContext
    from concourse.masks import make_identity

    f32 = mybir.dt.float32
    bf16 = mybir.dt.bfloat16
    EXP = mybir.ActivationFunctionType.Exp

    nc = bass.Bass()
    xg = nc.declare_dram_parameter("xg", [NB * P, NCH * BS], bf16, isOutput=False)
    wk = nc.declare_dram_parameter("wk", [P, NCH * P], bf16, isOutput=False)
    wv = nc.declare_dram_parameter("wv", [P, NCH * P], bf16, isOutput=False)
    wq = nc.declare_dram_parameter("wq", [P, NCH * P], bf16, isOutput=False)
    cs2 = nc.declare_dram_parameter("cs2", [P, T], bf16, isOutput=False)
    sn2 = nc.declare_dram_parameter("sn2", [P, T], bf16, isOutput=False)
    tailb = nc.declare_dram_parameter("tailb", [P, 1], f32, isOutput=False)
    oT = nc.declare_dram_parameter("oT", [P, 4 * BS], f32, isOutput=True)
    smv = nc.declare_dram_parameter("smv", [1, 4 * BS], f32, isOutput=True)

    NQ = 4  # x chunk-tiles per group (separate tiles -> fine-grained deps)
    CPQ = NCH // NQ  # c-chunks per x tile

    with TileContext(nc) as tc:
        with (
            tc.tile_pool(name="const", bufs=1) as cp,
            tc.tile_pool(name="store", bufs=1) as stp,
        ):
            wkt = cp.tile([P, NCH * P], bf16, tag="wkt")
            nc.sync.dma_start(wkt[:], wk[:])
            wvt = cp.tile([P, NCH * P], bf16, tag="wvt")
            nc.sync.dma_start(wvt[:], wv[:])
            wqt = cp.tile([P, NCH * P], bf16, tag="wqt")
            nc.sync.dma_start(wqt[:], wq[:])
            cst = cp.tile([P, T], bf16, tag="cst")
            nc.scalar.dma_start(cst[:], cs2[:])
            snt = cp.tile([P, T], bf16, tag="snt")
            nc.scalar.dma_start(snt[:], sn2[:])
            tb = cp.tile([P, 1], f32, tag="tb")
            nc.scalar.dma_start(tb[:], tailb[:])

            ident = cp.tile([P, P], bf16, tag="ident")
            make_identity(nc, ident[:])
            tri = cp.tile([P, P], bf16, tag="tri")
            nc.gpsimd.memset(tri[:], 0.0)
            nc.gpsimd.affine_select(
                out=tri[:], in_=tri[:],
                compare_op=mybir.AluOpType.is_gt,
                fill=1.0, base=0,
                pattern=[[-1, P]], channel_multiplier=1,
            )
            ones = cp.tile([P, 1], bf16, tag="ones")
            nc.gpsimd.memset(ones[:], 1.0)

            qT = stp.tile([P, 4 * BS], bf16, tag="qT")
            kT = stp.tile([P, T], bf16, tag="kT")
            vsb = stp.tile([P, T], bf16, tag="vsb")

            # ---- phase 1 ----
            with (
                tc.tile_pool(name="xp", bufs=2) as xp,
                tc.tile_pool(name="rp", bufs=2) as rp,
                tc.tile_pool(name="prj", bufs=2, space="PSUM") as prj,
            ):
                for g in range(NB):
                    gs = slice(g * BS, (g + 1) * BS)
                    xts = []
                    for q4 in range(NQ):
                        xt = xp.tile([P, CPQ * BS], bf16, tag=f"xg{q4}")
                        nc.sync.dma_start(
                            xt[:],
                            xg[g * P:(g + 1) * P,
                               q4 * CPQ * BS:(q4 + 1) * CPQ * BS])
                        xts.append(xt)

                    def proj(wt, tag):
                        pp = prj.tile([P, BS], f32, tag=tag)
                        for ci in range(NCH):
                            nc.tensor.matmul(
                                pp[:], wt[:, ci * P:(ci + 1) * P],
                                xts[ci // CPQ][:, (ci % CPQ) * BS:
                                               (ci % CPQ + 1) * BS],
                                start=(ci == 0), stop=(ci == NCH - 1))
                        return pp

                    def rope(pp, dst):
                        H = 64
                        m1 = rp.tile([P, BS], bf16, tag="m1")
                        nc.vector.tensor_mul(m1[:], pp[:], cst[:, gs])
                        rot = rp.tile([P, BS], bf16, tag="rot")
                        nc.vector.tensor_mul(rot[0:H, :], pp[H:P, :],
                                             snt[0:H, gs])
                        nc.vector.tensor_mul(rot[H:P, :], pp[0:H, :],
                                             snt[H:P, gs])
                        nc.vector.tensor_add(dst, m1[:], rot[:])

                    kp = proj(wkt, "kps")
                    rope(kp, kT[:, gs])
                    vp = proj(wvt, "vps")
                    vsbh = rp.tile([P, BS], bf16, tag="vsbh")
                    nc.scalar.copy(vsbh[:], vp[:])
                    vtp = prj.tile([P, BS], bf16, tag="vtp")
                    for k in range(4):
                        ks = slice(k * P, (k + 1) * P)
                        nc.tensor.transpose(vtp[:, ks], vsbh[:, ks], ident[:])
                        nc.scalar.copy(
                            vsb[:, g * BS + k * P:g * BS + (k + 1) * P],
                            vtp[:, ks])
                    if g < 4:
                        qp = proj(wqt, "qps")
                        rope(qp, qT[:, gs])

            # ---- phase 2 ----
            with (
                tc.tile_pool(name="pt", bufs=3) as ptp,
                tc.tile_pool(name="pac", bufs=2) as pap,
                tc.tile_pool(name="osb", bufs=2) as osb,
                tc.tile_pool(name="sps", bufs=3, space="PSUM") as sps,
                tc.tile_pool(name="o2ps", bufs=1, space="PSUM") as o2ps,
                tc.tile_pool(name="smps", bufs=1, space="PSUM") as smps,
            ):
                for j in range(4):
                    qsl = slice(j * BS, (j + 1) * BS)
                    o2 = o2ps.tile([P, BS], f32, tag="o2")
                    pac = pap.tile([P, BS], bf16, tag="pac")
                    pairs = ([(j, "diag", 0), (j, "diag", 2)]
                             + [(s, "full", st) for s in range(j)
                                for st in (0, 2)]
                             + [(4 + s, "full", st) for s in range(j)
                                for st in (0, 2)]
                             + [(4 + j, "tail", 0), (4 + j, "tail", 2)])
                    npair = len(pairs)
                    state = []

                    def emit_S(pi):
                        si, kind, st0 = pairs[pi]
                        Sps = sps.tile([P, 2 * BS], f32, tag="S")
                        Pt = ptp.tile([P, 2 * BS], bf16, tag="Pt")
                        halves = []
                        for h in range(2):
                            st = st0 + h
                            off = st * P if kind == "diag" else 0
                            w = BS - off
                            cb = h * BS
                            scol = si * BS + st * P
                            nc.tensor.matmul(
                                Sps[:, cb:cb + w], kT[:, scol:scol + P],
                                qT[:, j * BS + off:(j + 1) * BS],
                                start=True, stop=True)
                            halves.append((cb, off, w, scol))
                        state.append((Sps, Pt, kind, halves, pi))

                    def emit_rest():
                        Sps, Pt, kind, halves, pi = state.pop(0)
                        bias = tb[:, 0:1] if kind == "tail" else 0.0
                        if kind == "diag":
                            for (cb, off, w, scol) in halves:
                                nc.scalar.activation(
                                    Pt[:, cb:cb + w], Sps[:, cb:cb + w],
                                    EXP, bias=bias, scale=SCALE)
                        else:
                            nc.scalar.activation(
                                Pt[:, 0:2 * BS], Sps[:, 0:2 * BS],
                                EXP, bias=bias, scale=SCALE)
                        if kind == "diag":
                            for (cb, off, w, scol) in halves:
                                nc.vector.tensor_mul(
                                    Pt[:, cb:cb + P], Pt[:, cb:cb + P],
                                    tri[:])
                            if pi == 0:   # diag st0: full width init
                                nc.vector.tensor_copy(pac[:], Pt[:, 0:BS])
                                nc.vector.tensor_add(
                                    pac[:, P:BS], pac[:, P:BS],
                                    Pt[:, BS:BS + 384])
                            else:         # diag st2 pair: offsets 256/384
                                nc.vector.tensor_add(
                                    pac[:, 256:BS], pac[:, 256:BS],
                                    Pt[:, 0:256])
                                nc.vector.tensor_add(
                                    pac[:, 384:BS], pac[:, 384:BS],
                                    Pt[:, BS:BS + P])
                        else:
                            psum2 = pap.tile([P, BS], bf16, tag="psum2",
                                             bufs=2)
                            nc.gpsimd.tensor_add(
                                psum2[:], Pt[:, 0:BS], Pt[:, BS:2 * BS])
                            nc.vector.tensor_add(pac[:], pac[:], psum2[:])
                        for hi, (cb, off, w, scol) in enumerate(halves):
                            nc.tensor.matmul(
                                o2[:, off:BS], vsb[:, scol:scol + P],
                                Pt[:, cb:cb + w],
                                start=(pi == 0 and hi == 0),
                                stop=(pi == npair - 1 and hi == 1))

                    emit_S(0)
                    for pi in range(1, npair):
                        emit_S(pi)
                        emit_rest()
                    emit_rest()

                    sm = smps.tile([1, BS], f32, tag="sm")
                    nc.tensor.matmul(sm[:], ones[:], pac[:],
                                     start=True, stop=True)
                    o2sb = osb.tile([P, BS], f32, tag="o2sb")
                    nc.scalar.copy(o2sb[:], o2[:])
                    nc.sync.dma_start(oT[:, qsl], o2sb[:])
                    smsb = osb.tile([1, BS], f32, tag="smsb")
                    nc.scalar.copy(smsb[:], sm[:])
                    nc.sync.dma_start(smv[:, qsl], smsb[:])

    bass_rust.generate_event_semaphores(nc)
    return nc


_CACHE = {}


def _get_nc():
    if "nc" not in _CACHE:
        _CACHE["nc"] = build()
    return _CACHE["nc"]


def _prep_inputs(x, Wq, Wk, Wv, cos, sin):
    perm = np.concatenate([np.arange(0, HD, 2), np.arange(1, HD, 2)])

    def packw(wt):
        return np.ascontiguousarray(
            wt.reshape(NCH, P, HD).transpose(1, 0, 2).reshape(P, NCH * HD))

    wq = packw(Wq[perm].T.astype(BF))
    wk = packw(Wk[perm].T.astype(BF))
    wv = packw(Wv.T.astype(BF))
    cosT = cos.T.astype(np.float32)
    sinT = sin.T.astype(np.float32)
    cs2f = np.concatenate([cosT, cosT], axis=0)
    sn2f = np.concatenate([-sinT, sinT], axis=0)
    in_maps = []
    orders = []
    for c in range(8):
        b, par = c // 2, c % 2
        order = [par, par + 2, par + 4, par + 6,
                 1 - par, 3 - par, 5 - par, 7 - par]
        orders.append(order)
        xb = np.asarray(x[b], np.float32)
        xgl = np.empty((NB, P, NCH, BS), BF)
        c2 = np.empty((P, T), BF)
        s2 = np.empty((P, T), BF)
        for sl, ab in enumerate(order):
            seg = xb[ab * BS:(ab + 1) * BS].T.astype(BF)
            xgl[sl] = seg.reshape(NCH, P, BS).transpose(1, 0, 2)
            dst = slice(sl * BS, (sl + 1) * BS)
            src = slice(ab * BS, (ab + 1) * BS)
            c2[:, dst] = cs2f[:, src].astype(BF)
            s2[:, dst] = sn2f[:, src].astype(BF)
        tb = np.full((P, 1), NEG if par == 0 else 0.0, np.float32)
        in_maps.append({
            "xg": np.ascontiguousarray(xgl.reshape(NB * P, NCH * BS)),
            "wk": wk, "wv": wv, "wq": wq,
            "cs2": np.ascontiguousarray(c2),
            "sn2": np.ascontiguousarray(s2),
            "tailb": tb,
        })
    return in_maps, orders


def _run(x, Wq, Wk, Wv, cos, sin, trace=False):
    from concourse.bass_utils import run_bass_kernel_spmd
    nc = _get_nc()
    in_maps, orders = _prep_inputs(x, Wq, Wk, Wv, cos, sin)
    res = run_bass_kernel_spmd(nc, in_maps, list(range(8)), trace=trace)
    full = np.empty((B, T, HD), np.float32)
    for c in range(8):
        b, order = c // 2, orders[c]
        oc = res.results[c]["oT"]
        sc = res.results[c]["smv"]
        on = (oc / sc).T
        for j in range(4):
            ab = order[j]
            full[b, ab * BS:(ab + 1) * BS] = on[j * BS:(j + 1) * BS]
    return full, res


def kernel(x, Wq, Wk, Wv, cos, sin):
    return _run(x, Wq, Wk, Wv, cos, sin, trace=False)[0]


# revision 3
# speedup vs baseline: 1.0727x; 1.0727x over previous
"""Single-head causal attention with RoPE on 8 TRN2 NeuronCores.

Sharding: core c -> batch c//2, parity p = c%2 takes the interleaved
512-row q-blocks {p, p+2, p+4, p+6} of T=4096 (causal load balance).
Each core computes full K/V for its batch; no collectives.

vs the 214us fp32 baseline (now ~167-170us):
- All inputs bf16, x host-packed so each DMA is long contiguous runs.
- Q^T/K^T projected directly in [d, t] layout (weights-stationary
  matmuls, no PE transposes); RoPE applied in [d, t] on DVE via
  partition-offset rotate-half; V via PE transpose to [s, d].
- Phase 2: S^T scores with causal-exact shrinking diagonal tiles;
  exps batched over [128, 1024] PSUM pairs; S matmuls software-
  pipelined one pair ahead so the in-order PE queue never stalls
  behind the exp; probability row-sums accumulated in bf16 (GpSimd
  pair-sum + DVE accumulate) with one ones-matmul per q-block.
- Output written unnormalized [d, q] + row sums; final divide and
  transpose happen on the host.
"""
import numpy as np
import ml_dtypes

B, T, C, HD = 4, 4096, 2048, 128
P = 128
NB = 8
BS = 512
NCH = 16
SCALE = float(C) ** -0.5
NEG = -1.0e9
BF = ml_dtypes.bfloat16


def build():
    import concourse.bass as bass
    import concourse.mybir as mybir
    import bass_rust
    from concourse.tile import TileContext
    from concourse.masks import make_identity

    f32 = mybir.dt.float32
    bf16 = mybir.dt.bfloat16
    EXP = mybir.ActivationFunctionType.Exp

    nc = bass.Bass()
    xg = nc.declare_dram_parameter("xg", [NB * P, NCH * BS], bf16, isOutput=False)
    wk = nc.declare_dram_parameter("wk", [P, NCH * P], bf16, isOutput=False)
    wv = nc.declare_dram_parameter("wv", [P, NCH * P], bf16, isOutput=False)
    wq = nc.declare_dram_parameter("wq", [P, NCH * P], bf16, isOutput=False)
    cs2 = nc.declare_dram_parameter("cs2", [P, T], bf16, isOutput=False)
    sn2 = nc.declare_dram_parameter("sn2", [P, T], bf16, isOutput=False)
    tailb = nc.declare_dram_parameter("tailb", [P, 1], f32, isOutput=False)
    oT = nc.declare_dram_parameter("oT", [P, 4 * BS], f32, isOutput=True)
    smv = nc.declare_dram_parameter("smv", [1, 4 * BS], f32, isOutput=True)

    NQ = 4  # x chunk-tiles per group (separate tiles -> fine-grained deps)
    CPQ = NCH // NQ  # c-chunks per x tile

    with TileContext(nc) as tc:
        with (
            tc.tile_pool(name="const", bufs=1) as cp,
            tc.tile_pool(name="store", bufs=1) as stp,
        ):
            wkt = cp.tile([P, NCH * P], bf16, tag="wkt")
            nc.sync.dma_start(wkt[:], wk[:])
            wvt = cp.tile([P, NCH * P], bf16, tag="wvt")
            nc.sync.dma_start(wvt[:], wv[:])
            wqt = cp.tile([P, NCH * P], bf16, tag="wqt")
            nc.sync.dma_start(wqt[:], wq[:])
            cst = cp.tile([P, T], bf16, tag="cst")
            nc.scalar.dma_start(cst[:], cs2[:])
            snt = cp.tile([P, T], bf16, tag="snt")
            nc.scalar.dma_start(snt[:], sn2[:])
            tb = cp.tile([P, 1], f32, tag="tb")
            nc.scalar.dma_start(tb[:], tailb[:])

            ident = cp.tile([P, P], bf16, tag="ident")
            make_identity(nc, ident[:])
            tri = cp.tile([P, P], bf16, tag="tri")
            nc.gpsimd.memset(tri[:], 0.0)
            nc.gpsimd.affine_select(
                out=tri[:], in_=tri[:],
                compare_op=mybir.AluOpType.is_gt,
                fill=1.0, base=0,
                pattern=[[-1, P]], channel_multiplier=1,
            )
            ones = cp.tile([P, 1], bf16, tag="ones")
            nc.gpsimd.memset(ones[:], 1.0)

            qT = stp.tile([P, 4 * BS], bf16, tag="qT")
            kT = stp.tile([P, T], bf16, tag="kT")
            vsb = stp.tile([P, T], bf16, tag="vsb")

            # ---- phase 1 ----
            with (
                tc.tile_pool(name="xp", bufs=2) as xp,
                tc.tile_pool(name="rp", bufs=2) as rp,
                tc.tile_pool(name="prj", bufs=2, space="PSUM") as prj,
            ):
                for g in range(NB):
                    gs = slice(g * BS, (g + 1) * BS)
                    xts = []
                    for q4 in range(NQ):
                        xt = xp.tile([P, CPQ * BS], bf16, tag=f"xg{q4}")
                        nc.sync.dma_start(
                            xt[:],
                            xg[g * P:(g + 1) * P,
                               q4 * CPQ * BS:(q4 + 1) * CPQ * BS])
                        xts.append(xt)

                    def proj(wt, tag):
                        pp = prj.tile([P, BS], f32, tag=tag)
                        for ci in range(NCH):
                            nc.tensor.matmul(
                                pp[:], wt[:, ci * P:(ci + 1) * P],
                                xts[ci // CPQ][:, (ci % CPQ) * BS:
                                               (ci % CPQ + 1) * BS],
                                start=(ci == 0), stop=(ci == NCH - 1))
                        return pp

                    def rope(pp, dst):
                        H = 64
                        m1 = rp.tile([P, BS], bf16, tag="m1")
                        nc.vector.tensor_mul(m1[:], pp[:], cst[:, gs])
                        rot = rp.tile([P, BS], bf16, tag="rot")
                        nc.vector.tensor_mul(rot[0:H, :], pp[H:P, :],
                                             snt[0:H, gs])
                        nc.vector.tensor_mul(rot[H:P, :], pp[0:H, :],
                                             snt[H:P, gs])
                        nc.vector.tensor_add(dst, m1[:], rot[:])

                    kp = proj(wkt, "kps")
                    rope(kp, kT[:, gs])
                    vp = proj(wvt, "vps")
                    vsbh = rp.tile([P, BS], bf16, tag="vsbh")
                    nc.scalar.copy(vsbh[:], vp[:])
                    vtp = prj.tile([P, BS], bf16, tag="vtp")
                    for k in range(4):
                        ks = slice(k * P, (k + 1) * P)
                        nc.tensor.transpose(vtp[:, ks], vsbh[:, ks], ident[:])
                        nc.scalar.copy(
                            vsb[:, g * BS + k * P:g * BS + (k + 1) * P],
                            vtp[:, ks])
                    if g < 4:
                        qp = proj(wqt, "qps")
                        rope(qp, qT[:, gs])

            # ---- phase 2 ----
            with (
                tc.tile_pool(name="pt", bufs=3) as ptp,
                tc.tile_pool(name="pac", bufs=2) as pap,
                tc.tile_pool(name="osb", bufs=2) as osb,
                tc.tile_pool(name="sps", bufs=3, space="PSUM") as sps,
                tc.tile_pool(name="o2ps", bufs=1, space="PSUM") as o2ps,
                tc.tile_pool(name="smps", bufs=1, space="PSUM") as smps,
            ):
                for j in range(4):
                    qsl = slice(j * BS, (j + 1) * BS)
                    o2 = o2ps.tile([P, BS], f32, tag="o2")
                    pac = pap.tile([P, BS], bf16, tag="pac")
                    pairs = ([(j, "diag", 0), (j, "diag", 2)]
                             + [(s, "full", st) for s in range(j)
                                for st in (0, 2)]
                             + [(4 + s, "full", st) for s in range(j)
                                for st in (0, 2)]
                             + [(4 + j, "tail", 0), (4 + j, "tail", 2)])
                    npair = len(pairs)
                    state = []

                    def emit_S(pi):
                        si, kind, st0 = pairs[pi]
                        Sps = sps.tile([P, 2 * BS], f32, tag="S")
                        Pt = ptp.tile([P, 2 * BS], bf16, tag="Pt")
                        halves = []
                        for h in range(2):
                            st = st0 + h
                            off = st * P if kind == "diag" else 0
                            w = BS - off
                            cb = h * BS
                            scol = si * BS + st * P
                            nc.tensor.matmul(
                                Sps[:, cb:cb + w], kT[:, scol:scol + P],
                                qT[:, j * BS + off:(j + 1) * BS],
                                start=True, stop=True)
                            halves.append((cb, off, w, scol))
                        state.append((Sps, Pt, kind, halves, pi))

                    def emit_rest():
                        Sps, Pt, kind, halves, pi = state.pop(0)
                        bias = tb[:, 0:1] if kind == "tail" else 0.0
                        if kind == "diag":
                            for (cb, off, w, scol) in halves:
                                nc.scalar.activation(
                                    Pt[:, cb:cb + w], Sps[:, cb:cb + w],
                                    EXP, bias=bias, scale=SCALE)
                        else:
                            nc.scalar.activation(
                                Pt[:, 0:2 * BS], Sps[:, 0:2 * BS],
                                EXP, bias=bias, scale=SCALE)
                        if kind == "diag":
                            for (cb, off, w, scol) in halves:
                                nc.vector.tensor_mul(
                                    Pt[:, cb:cb + P], Pt[:, cb:cb + P],
                                    tri[:])
                            if pi == 0:   # diag st0: full width init
                                nc.vector.tensor_copy(pac[:], Pt[:, 0:BS])
                                nc.vector.tensor_add(
                                    pac[:, P:BS], pac[:, P:BS],
                                    Pt[:, BS:BS + 384])
                            else:         # diag st2 pair: offsets 256/384
                                nc.vector.tensor_add(
                                    pac[:, 256:BS], pac[:, 256:BS],
                                    Pt[:, 0:256])
                                nc.vector.tensor_add(
                                    pac[:, 384:BS], pac[:, 384:BS],
                                    Pt[:, BS:BS + P])
                        else:
                            psum2 = pap.tile([P, BS], bf16, tag="psum2",
                                             bufs=2)
                            nc.gpsimd.tensor_add(
                                psum2[:], Pt[:, 0:BS], Pt[:, BS:2 * BS])
                            nc.vector.tensor_add(pac[:], pac[:], psum2[:])
                        for hi, (cb, off, w, scol) in enumerate(halves):
                            nc.tensor.matmul(
                                o2[:, off:BS], vsb[:, scol:scol + P],
                                Pt[:, cb:cb + w],
                                start=(pi == 0 and hi == 0),
                                stop=(pi == npair - 1 and hi == 1))

                    emit_S(0)
                    for pi in range(1, npair):
                        emit_S(pi)
                        emit_rest()
                    emit_rest()

                    sm = smps.tile([1, BS], f32, tag="sm")
                    nc.tensor.matmul(sm[:], ones[:], pac[:],
                                     start=True, stop=True)
                    o2sb = osb.tile([P, BS], f32, tag="o2sb")
                    nc.scalar.copy(o2sb[:], o2[:])
                    nc.sync.dma_start(oT[:, qsl], o2sb[:])
                    smsb = osb.tile([1, BS], f32, tag="smsb")
                    nc.scalar.copy(smsb[:], sm[:])
                    nc.sync.dma_start(smv[:, qsl], smsb[:])

    bass_rust.generate_event_semaphores(nc)
    return nc


_CACHE = {}


def _get_nc():
    if "nc" not in _CACHE:
        _CACHE["nc"] = build()
    return _CACHE["nc"]


def _prep_inputs(x, Wq, Wk, Wv, cos, sin):
    perm = np.concatenate([np.arange(0, HD, 2), np.arange(1, HD, 2)])

    def packw(wt):
        return np.ascontiguousarray(
            wt.reshape(NCH, P, HD).transpose(1, 0, 2).reshape(P, NCH * HD))

    wq = packw(Wq[perm].T.astype(BF))
    wk = packw(Wk[perm].T.astype(BF))
    wv = packw(Wv.T.astype(BF))
    cosT = cos.T.astype(np.float32)
    sinT = sin.T.astype(np.float32)
    cs2f = np.concatenate([cosT, cosT], axis=0)
    sn2f = np.concatenate([-sinT, sinT], axis=0)
    in_maps = []
    orders = []
    for c in range(8):
        b, par = c // 2, c % 2
        order = [par, par + 2, par + 4, par + 6,
                 1 - par, 3 - par, 5 - par, 7 - par]
        orders.append(order)
        xb = np.asarray(x[b], np.float32)
        xgl = np.empty((NB, P, NCH, BS), BF)
        c2 = np.empty((P, T), BF)
        s2 = np.empty((P, T), BF)
        for sl, ab in enumerate(order):
            seg = xb[ab * BS:(ab + 1) * BS].T.astype(BF)
            xgl[sl] = seg.reshape(NCH, P, BS).transpose(1, 0, 2)
            dst = slice(sl * BS, (sl + 1) * BS)
            src = slice(ab * BS, (ab + 1) * BS)
            c2[:, dst] = cs2f[:, src].astype(BF)
            s2[:, dst] = sn2f[:, src].astype(BF)
        tb = np.full((P, 1), NEG if par == 0 else 0.0, np.float32)
        in_maps.append({
            "xg": np.ascontiguousarray(xgl.reshape(NB * P, NCH * BS)),
            "wk": wk, "wv": wv, "wq": wq,
            "cs2": np.ascontiguousarray(c2),
            "sn2": np.ascontiguousarray(s2),
            "tailb": tb,
        })
    return in_maps, orders


def _run(x, Wq, Wk, Wv, cos, sin, trace=False):
    from concourse.bass_utils import run_bass_kernel_spmd
    nc = _get_nc()
    in_maps, orders = _prep_inputs(x, Wq, Wk, Wv, cos, sin)
    res = run_bass_kernel_spmd(nc, in_maps, list(range(8)), trace=trace)
    full = np.empty((B, T, HD), np.float32)
    for c in range(8):
        b, order = c // 2, orders[c]
        oc = res.results[c]["oT"]
        sc = res.results[c]["smv"]
        on = (oc / sc).T
        for j in range(4):
            ab = order[j]
            full[b, ab * BS:(ab + 1) * BS] = on[j * BS:(j + 1) * BS]
    return full, res


def kernel(x, Wq, Wk, Wv, cos, sin):
    return _run(x, Wq, Wk, Wv, cos, sin, trace=False)[0]


# revision 4
# speedup vs baseline: 1.1819x; 1.1018x over previous
"""Single-head causal attention with RoPE on 8 TRN2 NeuronCores.

Sharding: core c -> batch c//2, parity p = c%2 takes the interleaved
512-row q-blocks {p, p+2, p+4, p+6} of T=4096 (causal load balance).
Each core computes full K/V for its batch; no collectives.

vs the 214us fp32 baseline (now ~159us):
- All inputs bf16, x host-packed into per-group contiguous layout;
  the first two groups are strip-split across DMA engines for a fast
  pipeline start (one dma_start is served by a single ~20GB/s engine).
- Q^T/K^T projected directly in [d, t] layout (weights-stationary
  matmuls, no PE transposes); RoPE applied in [d, t] on DVE via
  partition-offset rotate-half; V via PE transpose to [s, d].
- Phase 2: S^T scores with causal-exact shrinking diagonal tiles;
  exps batched over [128, 1024] PSUM pairs; S matmuls software-
  pipelined one pair ahead so the in-order PE queue never stalls
  behind the exp; probability row-sums accumulated in bf16 (pair-sum
  alternating GpSimd/DVE + DVE accumulate) with one ones-matmul per
  q-block; o2 PSUM->SBUF copies on DVE.
- Output written unnormalized [d, q] + row sums; final divide and
  transpose happen on the host.
"""
import numpy as np
import ml_dtypes

B, T, C, HD = 4, 4096, 2048, 128
P = 128
NB = 8
BS = 512
NCH = 16
SCALE = float(C) ** -0.5
NEG = -1.0e9
BF = ml_dtypes.bfloat16


def build():
    import concourse.bass as bass
    import concourse.mybir as mybir
    import bass_rust
    from concourse.tile import TileContext
    from concourse.masks import make_identity

    f32 = mybir.dt.float32
    bf16 = mybir.dt.bfloat16
    EXP = mybir.ActivationFunctionType.Exp

    nc = bass.Bass()
    xg = nc.declare_dram_parameter("xg", [NB * P, NCH * BS], bf16, isOutput=False)
    wk = nc.declare_dram_parameter("wk", [P, NCH * P], bf16, isOutput=False)
    wv = nc.declare_dram_parameter("wv", [P, NCH * P], bf16, isOutput=False)
    wq = nc.declare_dram_parameter("wq", [P, NCH * P], bf16, isOutput=False)
    cs2 = nc.declare_dram_parameter("cs2", [P, T], bf16, isOutput=False)
    sn2 = nc.declare_dram_parameter("sn2", [P, T], bf16, isOutput=False)
    tailb = nc.declare_dram_parameter("tailb", [P, 1], f32, isOutput=False)
    oT = nc.declare_dram_parameter("oT", [P, 4 * BS], f32, isOutput=True)
    smv = nc.declare_dram_parameter("smv", [1, 4 * BS], f32, isOutput=True)

    NQ = 4  # x chunk-tiles per group (separate tiles -> fine-grained deps)
    CPQ = NCH // NQ  # c-chunks per x tile

    with TileContext(nc) as tc:
        with (
            tc.tile_pool(name="const", bufs=1) as cp,
            tc.tile_pool(name="store", bufs=1) as stp,
        ):
            wkt = cp.tile([P, NCH * P], bf16, tag="wkt")
            nc.sync.dma_start(wkt[:], wk[:])
            wvt = cp.tile([P, NCH * P], bf16, tag="wvt")
            nc.sync.dma_start(wvt[:], wv[:])
            wqt = cp.tile([P, NCH * P], bf16, tag="wqt")
            nc.sync.dma_start(wqt[:], wq[:])
            cst = cp.tile([P, T], bf16, tag="cst")
            nc.scalar.dma_start(cst[:], cs2[:])
            snt = cp.tile([P, T], bf16, tag="snt")
            nc.scalar.dma_start(snt[:], sn2[:])
            tb = cp.tile([P, 1], f32, tag="tb")
            nc.scalar.dma_start(tb[:], tailb[:])

            ident = cp.tile([P, P], bf16, tag="ident")
            make_identity(nc, ident[:])
            tri = cp.tile([P, P], bf16, tag="tri")
            nc.gpsimd.memset(tri[:], 0.0)
            nc.gpsimd.affine_select(
                out=tri[:], in_=tri[:],
                compare_op=mybir.AluOpType.is_gt,
                fill=1.0, base=0,
                pattern=[[-1, P]], channel_multiplier=1,
            )
            ones = cp.tile([P, 1], bf16, tag="ones")
            nc.gpsimd.memset(ones[:], 1.0)

            qT = stp.tile([P, 4 * BS], bf16, tag="qT")
            kT = stp.tile([P, T], bf16, tag="kT")
            vsb = stp.tile([P, T], bf16, tag="vsb")

            # ---- phase 1 ----
            with (
                tc.tile_pool(name="xp", bufs=2) as xp,
                tc.tile_pool(name="rp", bufs=2) as rp,
                tc.tile_pool(name="prj", bufs=2, space="PSUM") as prj,
            ):
                for g in range(NB):
                    gs = slice(g * BS, (g + 1) * BS)
                    xts = []
                    for q4 in range(NQ):
                        xt = xp.tile([P, CPQ * BS], bf16, tag=f"xg{q4}")
                        c0 = q4 * CPQ * BS
                        if g < 2:
                            # strip-split across 4 DMA engines for a
                            # fast pipeline start
                            for s4 in range(4):
                                r0 = g * P + s4 * 32
                                eng = nc.sync if s4 % 2 == 0 else nc.scalar
                                eng.dma_start(
                                    xt[s4 * 32:(s4 + 1) * 32, :],
                                    xg[r0:r0 + 32, c0:c0 + CPQ * BS])
                        else:
                            nc.sync.dma_start(
                                xt[:], xg[g * P:(g + 1) * P,
                                          c0:c0 + CPQ * BS])
                        xts.append(xt)

                    def proj(wt, tag):
                        pp = prj.tile([P, BS], f32, tag=tag)
                        for ci in range(NCH):
                            nc.tensor.matmul(
                                pp[:], wt[:, ci * P:(ci + 1) * P],
                                xts[ci // CPQ][:, (ci % CPQ) * BS:
                                               (ci % CPQ + 1) * BS],
                                start=(ci == 0), stop=(ci == NCH - 1))
                        return pp

                    def rope(pp, dst):
                        H = 64
                        m1 = rp.tile([P, BS], bf16, tag="m1")
                        nc.vector.tensor_mul(m1[:], pp[:], cst[:, gs])
                        rot = rp.tile([P, BS], bf16, tag="rot")
                        nc.vector.tensor_mul(rot[0:H, :], pp[H:P, :],
                                             snt[0:H, gs])
                        nc.vector.tensor_mul(rot[H:P, :], pp[0:H, :],
                                             snt[H:P, gs])
                        nc.vector.tensor_add(dst, m1[:], rot[:])

                    kp = proj(wkt, "kps")
                    rope(kp, kT[:, gs])
                    vp = proj(wvt, "vps")
                    vsbh = rp.tile([P, BS], bf16, tag="vsbh")
                    nc.scalar.copy(vsbh[:], vp[:])
                    vtp = prj.tile([P, BS], bf16, tag="vtp")
                    for k in range(4):
                        ks = slice(k * P, (k + 1) * P)
                        nc.tensor.transpose(vtp[:, ks], vsbh[:, ks], ident[:])
                        nc.scalar.copy(
                            vsb[:, g * BS + k * P:g * BS + (k + 1) * P],
                            vtp[:, ks])
                    if g < 4:
                        qp = proj(wqt, "qps")
                        rope(qp, qT[:, gs])

            # ---- phase 2 ----
            with (
                tc.tile_pool(name="pt", bufs=3) as ptp,
                tc.tile_pool(name="pac", bufs=2) as pap,
                tc.tile_pool(name="osb", bufs=2) as osb,
                tc.tile_pool(name="sps", bufs=3, space="PSUM") as sps,
                tc.tile_pool(name="o2ps", bufs=1, space="PSUM") as o2ps,
                tc.tile_pool(name="smps", bufs=1, space="PSUM") as smps,
            ):
                for j in range(4):
                    qsl = slice(j * BS, (j + 1) * BS)
                    o2 = o2ps.tile([P, BS], f32, tag="o2")
                    pac = pap.tile([P, BS], bf16, tag="pac")
                    pairs = ([(j, "diag", 0), (j, "diag", 2)]
                             + [(s, "full", st) for s in range(j)
                                for st in (0, 2)]
                             + [(4 + s, "full", st) for s in range(j)
                                for st in (0, 2)]
                             + [(4 + j, "tail", 0), (4 + j, "tail", 2)])
                    npair = len(pairs)
                    state = []

                    def emit_S(pi):
                        si, kind, st0 = pairs[pi]
                        Sps = sps.tile([P, 2 * BS], f32, tag="S")
                        Pt = ptp.tile([P, 2 * BS], bf16, tag="Pt")
                        halves = []
                        for h in range(2):
                            st = st0 + h
                            off = st * P if kind == "diag" else 0
                            w = BS - off
                            cb = h * BS
                            scol = si * BS + st * P
                            nc.tensor.matmul(
                                Sps[:, cb:cb + w], kT[:, scol:scol + P],
                                qT[:, j * BS + off:(j + 1) * BS],
                                start=True, stop=True)
                            halves.append((cb, off, w, scol))
                        state.append((Sps, Pt, kind, halves, pi))

                    def emit_rest():
                        Sps, Pt, kind, halves, pi = state.pop(0)
                        bias = tb[:, 0:1] if kind == "tail" else 0.0
                        if kind == "diag":
                            for (cb, off, w, scol) in halves:
                                nc.scalar.activation(
                                    Pt[:, cb:cb + w], Sps[:, cb:cb + w],
                                    EXP, bias=bias, scale=SCALE)
                        else:
                            nc.scalar.activation(
                                Pt[:, 0:2 * BS], Sps[:, 0:2 * BS],
                                EXP, bias=bias, scale=SCALE)
                        if kind == "diag":
                            for (cb, off, w, scol) in halves:
                                nc.vector.tensor_mul(
                                    Pt[:, cb:cb + P], Pt[:, cb:cb + P],
                                    tri[:])
                            if pi == 0:   # diag st0: full width init
                                nc.vector.tensor_copy(pac[:], Pt[:, 0:BS])
                                nc.vector.tensor_add(
                                    pac[:, P:BS], pac[:, P:BS],
                                    Pt[:, BS:BS + 384])
                            else:         # diag st2 pair: offsets 256/384
                                nc.vector.tensor_add(
                                    pac[:, 256:BS], pac[:, 256:BS],
                                    Pt[:, 0:256])
                                nc.vector.tensor_add(
                                    pac[:, 384:BS], pac[:, 384:BS],
                                    Pt[:, BS:BS + P])
                        else:
                            psum2 = pap.tile([P, BS], bf16, tag="psum2",
                                             bufs=2)
                            peng = nc.gpsimd if pi % 2 == 0 else nc.vector
                            peng.tensor_add(
                                psum2[:], Pt[:, 0:BS], Pt[:, BS:2 * BS])
                            nc.vector.tensor_add(pac[:], pac[:], psum2[:])
                        for hi, (cb, off, w, scol) in enumerate(halves):
                            nc.tensor.matmul(
                                o2[:, off:BS], vsb[:, scol:scol + P],
                                Pt[:, cb:cb + w],
                                start=(pi == 0 and hi == 0),
                                stop=(pi == npair - 1 and hi == 1))

                    emit_S(0)
                    for pi in range(1, npair):
                        emit_S(pi)
                        emit_rest()
                    emit_rest()

                    sm = smps.tile([1, BS], f32, tag="sm")
                    nc.tensor.matmul(sm[:], ones[:], pac[:],
                                     start=True, stop=True)
                    o2sb = osb.tile([P, BS], f32, tag="o2sb")
                    nc.vector.tensor_copy(o2sb[:], o2[:])
                    nc.sync.dma_start(oT[:, qsl], o2sb[:])
                    smsb = osb.tile([1, BS], f32, tag="smsb")
                    nc.scalar.copy(smsb[:], sm[:])
                    nc.sync.dma_start(smv[:, qsl], smsb[:])

    bass_rust.generate_event_semaphores(nc)
    return nc


_CACHE = {}


def _get_nc():
    if "nc" not in _CACHE:
        _CACHE["nc"] = build()
    return _CACHE["nc"]


def _prep_inputs(x, Wq, Wk, Wv, cos, sin):
    perm = np.concatenate([np.arange(0, HD, 2), np.arange(1, HD, 2)])

    def packw(wt):
        return np.ascontiguousarray(
            wt.reshape(NCH, P, HD).transpose(1, 0, 2).reshape(P, NCH * HD))

    wq = packw(Wq[perm].T.astype(BF))
    wk = packw(Wk[perm].T.astype(BF))
    wv = packw(Wv.T.astype(BF))
    cosT = cos.T.astype(np.float32)
    sinT = sin.T.astype(np.float32)
    cs2f = np.concatenate([cosT, cosT], axis=0)
    sn2f = np.concatenate([-sinT, sinT], axis=0)
    in_maps = []
    orders = []
    for c in range(8):
        b, par = c // 2, c % 2
        order = [par, par + 2, par + 4, par + 6,
                 1 - par, 3 - par, 5 - par, 7 - par]
        orders.append(order)
        xb = np.asarray(x[b], np.float32)
        xgl = np.empty((NB, P, NCH, BS), BF)
        c2 = np.empty((P, T), BF)
        s2 = np.empty((P, T), BF)
        for sl, ab in enumerate(order):
            seg = xb[ab * BS:(ab + 1) * BS].T.astype(BF)
            xgl[sl] = seg.reshape(NCH, P, BS).transpose(1, 0, 2)
            dst = slice(sl * BS, (sl + 1) * BS)
            src = slice(ab * BS, (ab + 1) * BS)
            c2[:, dst] = cs2f[:, src].astype(BF)
            s2[:, dst] = sn2f[:, src].astype(BF)
        tb = np.full((P, 1), NEG if par == 0 else 0.0, np.float32)
        in_maps.append({
            "xg": np.ascontiguousarray(xgl.reshape(NB * P, NCH * BS)),
            "wk": wk, "wv": wv, "wq": wq,
            "cs2": np.ascontiguousarray(c2),
            "sn2": np.ascontiguousarray(s2),
            "tailb": tb,
        })
    return in_maps, orders


def _run(x, Wq, Wk, Wv, cos, sin, trace=False):
    from concourse.bass_utils import run_bass_kernel_spmd
    nc = _get_nc()
    in_maps, orders = _prep_inputs(x, Wq, Wk, Wv, cos, sin)
    res = run_bass_kernel_spmd(nc, in_maps, list(range(8)), trace=trace)
    full = np.empty((B, T, HD), np.float32)
    for c in range(8):
        b, order = c // 2, orders[c]
        oc = res.results[c]["oT"]
        sc = res.results[c]["smv"]
        on = (oc / sc).T
        for j in range(4):
            ab = order[j]
            full[b, ab * BS:(ab + 1) * BS] = on[j * BS:(j + 1) * BS]
    return full, res


def kernel(x, Wq, Wk, Wv, cos, sin):
    return _run(x, Wq, Wk, Wv, cos, sin, trace=False)[0]
